# revision 33
# baseline (speedup 1.0000x reference)
"""ChannelSELayer (global-avg-pool -> MLP -> sigmoid -> top-2 channel gather).

Full-input contract: kernel(**inputs) takes the complete tensors and returns
the complete output. Internally shards across 8 NeuronCores:

  core i -> (batch b = i//2, channel half h = i%2, output rank r = i%2)

Per core:
  1. Stream the 32 MiB own channel-half slab [128, 65536]:
       - SP + ACT issue plain DMA tiles        (1.542 ns/col each)
       - Pool issues dma_gather tiles          (0.833 ns/col)
     PE reduces every tile with fp32r selection-matmuls (sel.T @ tile)
     accumulating per-channel partial sums in PSUM (0.417 ns/col);
     a few prefix tiles go to otherwise-idle DVE (dedicated slots) to
     keep PE off the critical path.
  2. epsilon-split exchange: the pair AllGather (flat ~15 us on Pool)
     carries only the PREFIX portion of the channel sums, issued as soon
     as the prefix columns are reduced.  Both cores redundantly stream +
     reduce the peer half's TAIL columns locally (each core holds the
     full batch image xq), hiding the collective latency behind tail
     streaming.
  3. MLP layer 1 as three accumulating PE matmuls against host-prepared
     G matrices (1/S scaling and b1 folded in); LeakyReLU on DVE; layer 2
     matmul; DVE top-8/rank-select; ixv = 2048 * chosen channel.
  4. The selected channel (1 MiB) is copied by two dynamic-offset DMAs
     (SP + ACT, column-sliced so the priced APs stay 2-D at the 500 ns
     descriptor floor).

All cross-core communication is the single pair AllGather; routing-based
remote DMA is unavailable in this runtime.
"""

import numpy as np
from contextlib import ExitStack

import concourse.bass as bass
import concourse.bass_isa as bass_isa
import concourse.mybir as mybir
from concourse import library_config
from concourse.bass_utils import run_bass_kernel_spmd

F32 = mybir.dt.float32
F32R = mybir.dt.float32r
I16 = mybir.dt.int16
U32 = mybir.dt.uint32

B = 4
C = 64
S = 64 * 64 * 64
R = 2
N_CORES = 8
LEAKY_SLOPE = 0.01

PP = S // 4          # slab columns: x[b] viewed as [256, PP]
TW = 2048            # stream tile width (columns)
CHUNK = 512          # matmul moving width (one PSUM bank)

# --- tunables -----------------------------------------------------------
import os as _os
N_PREF = int(_os.environ.get("K_NPREF", 26))   # own-slab prefix 2048-blocks
N_POOL_TAIL = int(_os.environ.get("K_NPOOLTAIL", 1))
N_DVE = int(_os.environ.get("K_NDVE", 4))      # prefix tiles reduced on DVE
N_POOL_POST = int(_os.environ.get("K_POOLPOST", 0))  # tail gathers between bounce and CC
NOGATHER = int(_os.environ.get("K_NOGATHER", 1))     # pool uses plain DMAs (device-safe)
FP32MM = int(_os.environ.get("K_FP32MM", 0))         # plain-f32 reduce matmuls (debug)
DEBUG_OUT = int(_os.environ.get("K_DEBUG", 0))       # extra debug outputs
NSLOT = {"sp": 5, "act": 5, "pool": 5}
N_DVE_SLOT = max(N_DVE, 1)                     # one slot per DVE tile

# cost-model rates used only for schedule prediction (host side)
_PLAIN_DELAY = 1717.0
_GATH_COST = TW * 0.8333
_POOL_COST = _GATH_COST  # overridden below when NOGATHER
if NOGATHER:
    _POOL_COST = TW * 4 * 0.3855
_GATH_DELAY = 100.0
_CONST_COST = 500.0


def _plain_cost(w):
    return max(500.0, w * 4 * 0.3855)


def _tile_range(lo, cols, warm=False, taper=True):
    """Cut [lo, lo+cols) into tiles: optional small warmup tiles at the
    start, 2048s in the middle, small tapered tiles at the end (the last
    arrival then carries only ~790 ns of DMA cost after the issue end)."""
    tiles = []
    end = lo + cols
    if warm and cols >= 4096:
        tiles += [(lo, 512), (lo + 512, 512), (lo + 1024, 1024)]
        lo += 2048
    if not taper:
        assert lo % 2048 == 0 and (end - lo) % 2048 == 0
        while lo < end:
            tiles.append((lo, 2048))
            lo += 2048
        return tiles
    while end - lo > 4096:
        tiles.append((lo, 2048))
        lo += 2048
    rem = end - lo
    if rem > 2048:
        tiles.append((lo, rem - 2048))
        lo += rem - 2048
        rem = 2048
    if rem > 1024:
        tiles.append((lo, rem - 1024))
        tiles.append((lo + rem - 1024, 512))
        tiles.append((lo + rem - 512, 512))
    elif rem:
        tiles.append((lo, rem))
    return tiles


def _schedule():
    """Assign column quotas to engines, tile each quota with tapered ends.

    Returns (eng_prog, dve_recs, pe_order):
      eng_prog[eng] = list of ("pe", k, rec) / ("dve", dve_idx, rec) in
        issue order; k is the engine's pe-fill index.
      dve_recs = tile recs in DVE consumption order.
      pe_order = [(eng, k, rec)] — chain A tiles first, then C, then B
        (within each chain by predicted arrival).  Per-engine consumption
        stays in fill order because engines fill A, then C, then B.

    Tile rec: (chain, src, lo, w), src in {'xr','xp','pq'}.
    """
    P = N_PREF * 2048
    T = PP - P                      # own-tail cols = peer-tail cols

    # --- prefix quotas: balance last-ARRIVAL across engines ------------
    # pool arrival ~ t0p + q*0.8333 + 100 ; sp/act ~ t0 + q*1.542 + 2507
    best = None
    for qp_t in range(0, P // 2048 + 1):
        qp = qp_t * 2048
        rest = P - qp
        qs = (rest // 2 + 256) // 512 * 512
        qa = rest - qs
        prate = (4 * 0.3855) if NOGATHER else 0.8333
        pdel = (1717.0 + 790.0) if NOGATHER else 100.0
        a_p = 2317.0 + qp * prate + pdel if qp else 0.0
        a_s = 600.0 + qs * 1.542 + 1717.0 + 790.0
        a_a = 100.0 + qa * 1.542 + 1717.0 + 790.0
        m = max(a_p, a_s, a_a)
        if best is None or m < best[0]:
            best = (m, qp, qs, qa)
    _, qp, qs, qa = best

    # --- tail quotas: pool takes N_POOL_TAIL gathers, SP/ACT split the
    # rest of C (peer) then B (own); C is processed first by both so the
    # C chain closes early.
    pool_tail = N_POOL_TAIL * 2048
    rest = 2 * T - pool_tail
    ts = (rest // 2 + 256) // 512 * 512
    ta = rest - ts

    prog = {"sp": [], "act": [], "pool": []}
    nfill = {"sp": 0, "act": 0, "pool": 0}
    dve_recs = []
    t = {"sp": 100.0 + _CONST_COST, "act": 100.0, "pool": 2317.0}
    arrivals = []

    def put(eng, rec, kind="pe"):
        if eng == "pool":
            t[eng] += _POOL_COST
            arr = t[eng] + (_PLAIN_DELAY if NOGATHER else _GATH_DELAY)
        else:
            t[eng] += _plain_cost(rec[3])
            arr = t[eng] + _PLAIN_DELAY
        if kind == "dve":
            prog[eng].append(("dve", len(dve_recs), rec))
            dve_recs.append(rec)
        else:
            prog[eng].append(("pe", nfill[eng], rec))
            arrivals.append((arr, eng, nfill[eng], rec))
            nfill[eng] += 1

    # prefix: sp gets [0, qs), act [qs, qs+qa), pool [qs+qa, P)
    n_dve = 0
    for eng, lo0, q, warm in (("sp", 0, qs, True), ("act", qs, qa, True)):
        reg = 0
        for (l, w) in _tile_range(lo0, q, warm=warm):
            if w == 2048 and n_dve < N_DVE and 1 <= reg:
                put(eng, ("A", "xr", l, w), kind="dve")
                n_dve += 1
                reg = 0
            else:
                put(eng, ("A", "xr", l, w))
                reg += 1
    for (l, w) in _tile_range(qs + qa, qp, taper=bool(NOGATHER)):
        put("pool", ("A", "pq", l, w))

    # pool tail gathers: front of the peer-tail range
    for g in range(N_POOL_TAIL):
        put("pool", ("C", "pq", P + g * 2048, 2048))
    # SP/ACT: C remainder then B, each quota tapered at its end
    c_lo = P + pool_tail
    c_rest = T - pool_tail
    cs = min(ts, (c_rest // 2 + 256) // 512 * 512)
    for eng, lo0, q in (("sp", c_lo, cs), ("act", c_lo + cs, c_rest - cs)):
        for (l, w) in _tile_range(lo0, q):
            put(eng, ("C", "xp", l, w))
    bs = ts - cs
    for eng, lo0, q in (("sp", P, bs), ("act", P + bs, T - bs)):
        for (l, w) in _tile_range(lo0, q):
            put(eng, ("B", "xr", l, w))

    order = {"A": 0, "C": 1, "B": 2}
    arrivals.sort(key=lambda a: (order[a[3][0]], a[0]))
    pe_order = [(eng, k, rec) for _, eng, k, rec in arrivals]
    return prog, dve_recs, pe_order


ENG_PROG, DVE_RECS, PE_ORDER = _schedule()
N_DVE_ACT = len(DVE_RECS)
POOL_GATHERS = [e for e in ENG_PROG["pool"] if e[0] == "pe"]
N_GATH = len(POOL_GATHERS)

# const_sb column layout (f32 columns)
_G_RX = 0          # [65, 64]  G_rx_aug (rows 0..63: W1[:,j]/S ; row 64: b1)
_G_OWN = 64        # [32, 64]
_G_PEER = 128      # [32, 64]
_W2TA = 192        # [65, 64]  [W2.T ; b2]
_SEL = 256         # [128, 32] quarter-fold selection (f32 bits used as f32r)
NCONST = 288       # f32r-loaded numeric constants
# integer constants go through a separate plain-f32 DMA: the device's
# f32r DMA path rounds (flushes denormals), which would zero int payloads
_GIDX = 0          # [128, 4*N_GATH]  int16 gather indices (4 f32 cols each)
_RSEL = _GIDX + 4 * N_GATH   # [1, 8] uint32 rank select (2048*onehot)
NICONST = _RSEL + 8


def build_bass(n_cores=N_CORES, gather_bounds="skip_entire_dma"):
    nc = bass.Bass(num_devices=n_cores)
    xr = nc.declare_dram_parameter("xr", [128, PP], F32, isOutput=False)
    xp = nc.declare_dram_parameter("xp", [128, PP], F32, isOutput=False)
    xq = nc.declare_dram_parameter("xq", [256, PP], F32, isOutput=False)
    cst = nc.declare_dram_parameter("cst", [128, NCONST], F32, isOutput=False)
    icst = nc.declare_dram_parameter("icst", [128, NICONST], F32, isOutput=False)
    out = nc.declare_dram_parameter("out", [S], F32, isOutput=True)
    dbg = nc.declare_dram_parameter("dbg", [1, 512], F32, isOutput=True) if DEBUG_OUT else None

    part_dram = nc.dram_tensor("part_bounce", [1, 32], F32)
    full_dram = nc.dram_tensor("full_bounce", [1, 64], F32)

    xqv = xq.rearrange("r (g f) -> (r g) f", f=TW)     # [8192, 2048]
    xqg = xq.rearrange("r (g f) -> (r g) f", f=128)    # [131072, 128]
    ogv = out.rearrange("(p f) -> p f", f=128)         # [2048, 128]

    with ExitStack() as ctx:
        ent = ctx.enter_context
        sl_sp = [ent(nc.sbuf_tensor(f"sp{i}", [128, TW], F32)) for i in range(NSLOT["sp"])]
        sl_act = [ent(nc.sbuf_tensor(f"ac{i}", [128, TW], F32)) for i in range(NSLOT["act"])]
        sl_pool = [ent(nc.sbuf_tensor(f"pl{i}", [128, TW], F32)) for i in range(NSLOT["pool"])]
        sl_dve = [ent(nc.sbuf_tensor(f"dv{i}", [128, TW], F32)) for i in range(N_DVE_SLOT)]
        cst_sb = ent(nc.sbuf_tensor("cst_sb", [128, NCONST], F32R))
        icst_sb = ent(nc.sbuf_tensor("icst_sb", [128, NICONST], F32))
        dacc = ent(nc.sbuf_tensor("dacc", [128, max(N_DVE_ACT, 1)], F32))
        dcol = ent(nc.sbuf_tensor("dcol", [128, 1], F32))
        partA = ent(nc.sbuf_tensor("partA", [32, 1], F32))
        fB = ent(nc.sbuf_tensor("fB", [32, 1], F32))
        fC = ent(nc.sbuf_tensor("fC", [32, 1], F32))
        rx_aug = ent(nc.sbuf_tensor("rx_aug", [65, 1], F32))
        h_aug = ent(nc.sbuf_tensor("h_aug", [65, 1], F32))
        h_sc = ent(nc.sbuf_tensor("h_sc", [64, 1], F32))
        s_sb = ent(nc.sbuf_tensor("s_sb", [1, 64], F32))
        mx8 = ent(nc.sbuf_tensor("mx8", [1, 8], F32))
        ix8 = ent(nc.sbuf_tensor("ix8", [1, 8], U32))
        ixm = ent(nc.sbuf_tensor("ixm", [1, 8], U32))
        ixv = ent(nc.sbuf_tensor("ixv", [1, 1], U32))

        psA = ent(nc.psum_tensor([32, CHUNK], F32))
        psB = ent(nc.psum_tensor([32, CHUNK], F32))
        psC = ent(nc.psum_tensor([32, CHUNK], F32))
        ps1 = ent(nc.psum_tensor([64, 1], F32))
        ps2 = ent(nc.psum_tensor([1, 64], F32))
        psm = {"A": psA, "B": psB, "C": psC}

        s_const = ent(nc.semaphore("s_const"))
        s_fill_sp = [ent(nc.semaphore(f"s_fsp{i}")) for i in range(NSLOT["sp"])]
        s_fill_act = [ent(nc.semaphore(f"s_fac{i}")) for i in range(NSLOT["act"])]
        s_fill_pool = [ent(nc.semaphore(f"s_fpl{i}")) for i in range(NSLOT["pool"])]
        s_fdve = [ent(nc.semaphore(f"s_fdve{i}")) for i in range(N_DVE_SLOT)]
        s_cons = {k: ent(nc.semaphore(f"s_cons_{k}")) for k in ("sp", "act", "pool")}
        s_cdve = ent(nc.semaphore("s_cdve"))
        s_dacc = ent(nc.semaphore("s_dacc"))
        s_aclose = ent(nc.semaphore("s_aclose"))
        s_fA = ent(nc.semaphore("s_fA"))
        s_fB = ent(nc.semaphore("s_fB"))
        s_fC = ent(nc.semaphore("s_fC"))
        s_part = ent(nc.semaphore("s_part"))
        s_cc = ent(nc.semaphore("s_cc"))
        s_rx = ent(nc.semaphore("s_rx"))
        s_ms = ent(nc.semaphore("s_ms"))
        s_pe1 = ent(nc.semaphore("s_pe1"))
        s_haug = ent(nc.semaphore("s_haug"))
        s_pe2 = ent(nc.semaphore("s_pe2"))
        s_top = ent(nc.semaphore("s_top"))
        s_out = ent(nc.semaphore("s_out"))
        s_out2 = ent(nc.semaphore("s_out2"))
        block = ent(nc.Block())

        sel_ap = cst_sb[:, _SEL:_SEL + 32].bitcast(F32) if FP32MM else cst_sb[:, _SEL:_SEL + 32]
        sel_f32 = cst_sb[:, _SEL:_SEL + 32].bitcast(F32)
        g_rx = cst_sb[0:65, _G_RX:_G_RX + 64].bitcast(F32)
        g_own = cst_sb[0:32, _G_OWN:_G_OWN + 64].bitcast(F32)
        g_peer = cst_sb[0:32, _G_PEER:_G_PEER + 64].bitcast(F32)
        w2ta = cst_sb[0:65, _W2TA:_W2TA + 64].bitcast(F32)
        rsel = icst_sb[0:1, _RSEL:_RSEL + 8].bitcast(U32)

        slots = {"sp": sl_sp, "act": sl_act, "pool": sl_pool}
        fills = {"sp": s_fill_sp, "act": s_fill_act, "pool": s_fill_pool}

        def stream_plain(eng, key):
            ns = NSLOT[key]
            for kind, idx, (chain, src, lo, w) in ENG_PROG[key]:
                srct = xr if src == "xr" else xp
                if kind == "dve":
                    j = idx % N_DVE_SLOT
                    if idx >= N_DVE_SLOT:
                        eng.wait_ge(s_cdve, idx - N_DVE_SLOT + 1)
                    eng.dma_start(
                        sl_dve[j][:, 0:w], srct[:, lo:lo + w]
                    ).then_inc(s_fdve[j], 16)
                else:
                    if idx >= ns:
                        eng.wait_ge(s_cons[key], idx - ns + 1)
                    eng.dma_start(
                        slots[key][idx % ns][:, 0:w].bitcast(F32R),
                        srct[:, lo:lo + w].bitcast(F32R),
                    ).then_inc(fills[key][idx % ns], 16)

        def final_gather(eng, engine_type, cols, sem):
            reg = nc.values_load(
                ixv[0:1, 0:1], engines=[engine_type],
                min_val=0, max_val=(C - 1) * 2048,
                skip_runtime_bounds_check=True,
            )
            # column-sliced so the APs stay 2-D (a merged contiguous run
            # would be priced as 64 KiB-descriptor free bytes)
            eng.dma_start(
                ogv[:, cols[0]:cols[1]], xqg[bass.ds(reg, 2048), cols[0]:cols[1]],
                bounds_check=gather_bounds,
            ).then_inc(sem, 16)

        @block.sync
        def _(sync):
            sync.dma_start(cst_sb[:], cst[:].bitcast(F32R)).then_inc(s_const, 16)
            sync.dma_start(icst_sb[:], icst[:]).then_inc(s_const, 16)
            stream_plain(sync, "sp")
            sync.wait_ge(s_top, 1)
            final_gather(sync, mybir.EngineType.SP, (0, 64), s_out)
            if DEBUG_OUT:
                with nc.allow_non_contiguous_dma(reason="debug"):
                    sync.dma_start(dbg.rearrange("a f -> f a")[0:32], partA[:]).then_inc(s_out, 16)
                    sync.dma_start(dbg.rearrange("a f -> f a")[64:129], rx_aug[:]).then_inc(s_out, 16)
                    sync.dma_start(dbg[0:1, 160:224], s_sb[:]).then_inc(s_out, 16)
                    sync.dma_start(dbg[0:1, 224:225], ixv[:].bitcast(F32)).then_inc(s_out, 16)
                    sync.dma_start(dbg.rearrange("a f -> f a")[256:288], fB[:]).then_inc(s_out, 16)
                    sync.dma_start(dbg.rearrange("a f -> f a")[288:320], fC[:]).then_inc(s_out, 16)
                sync.wait_ge(s_out, 112)
            else:
                sync.wait_ge(s_out, 16)

        @block.scalar
        def _(scalar):
            stream_plain(scalar, "act")
            scalar.wait_ge(s_top, 1)
            final_gather(scalar, mybir.EngineType.Activation, (64, 128), s_out2)
            scalar.wait_ge(s_out2, 16)

        @block.gpsimd
        def _(gp):
            # encoded PSEUDO_LIBRARY_RELOAD_INDEX: CoreSim dispatches on the
            # typed class; walrus codegen needs the 64-byte ISA encoding
            # (bass's load_library leaves instr empty and fails codegen)
            gp.add_instruction(bass_isa.InstPseudoReloadLibraryIndex(
                name=nc.get_next_instruction_name(),
                ins=[], outs=[],
                lib_index=library_config.mlp.index,
                instr=bass_isa.isa_struct(
                    nc.isa, 223,
                    {"pseudo_opcode": 2,
                     "lib_index": library_config.mlp.index},
                    "NEURON_ISA_TPB_PSEUDO_LIBRARY_RELOAD_INDEX_STRUCT",
                )[0],
            ))
            gp.wait_ge(s_const, 32)
            ns = NSLOT["pool"]
            n_pre = len(POOL_GATHERS) - min(N_POOL_POST, N_POOL_TAIL)

            def emit_gather(g, idx, chain, lo, w):
                if idx >= ns:
                    gp.wait_ge(s_cons["pool"], idx - ns + 1)
                if NOGATHER:
                    srct = xr if chain in ("A", "B") else xp
                    gp.dma_start(
                        sl_pool[idx % ns][:, 0:w].bitcast(F32R),
                        srct[:, lo:lo + w].bitcast(F32R),
                    ).then_inc(s_fill_pool[idx % ns], 16)
                    return
                gidx = icst_sb[:, _GIDX + 4 * g:_GIDX + 4 * (g + 1)].bitcast(I16)
                gp.dma_gather(
                    sl_pool[idx % ns][:].rearrange("p (a e) -> p a e", a=1)
                    .bitcast(F32R),
                    xqv[:].bitcast(F32R),
                    gidx,
                    num_idxs=128,
                    num_idxs_reg=128,
                    elem_size=TW,
                ).then_inc(s_fill_pool[idx % ns], 16)

            for g, (kind, idx, (chain, src, lo, w)) in enumerate(POOL_GATHERS[:n_pre]):
                emit_gather(g, idx, chain, lo, w)
            # CC input bounce; the remaining tail gathers fill the DGE
            # latency window between the bounce issue and the CC start
            gp.wait_ge(s_fA, 1)
            with nc.allow_non_contiguous_dma(reason="32 4-byte descriptors"):
                gp.dma_start(
                    part_dram.rearrange("a p -> p a"), partA[:]
                ).then_inc(s_part, 16)
            for g, (kind, idx, (chain, src, lo, w)) in enumerate(POOL_GATHERS[n_pre:]):
                emit_gather(n_pre + g, idx, chain, lo, w)
            gp.wait_ge(s_part, 16)
            groups = [[i, i + 1] for i in range(0, n_cores, 2)]
            gp.collective_compute(
                "AllGather",
                mybir.AluOpType.bypass,
                replica_groups=groups,
                ins=[part_dram[:]],
                outs=[full_dram[:]],
            ).then_inc(s_cc, 1)
            gp.wait_ge(s_cc, 1)
            with nc.allow_non_contiguous_dma(reason="64 4-byte descriptors"):
                gp.dma_start(
                    rx_aug[0:64, 0:1], full_dram.rearrange("a p -> p a")
                ).then_inc(s_rx, 16)

        # chain -> (engine-consumption sem key, count) of its closing matmul
        chain_close = {}

        @block.tensor
        def _(tensor):
            tensor.wait_ge(s_const, 32)
            tensor.wait_ge(s_ms, 2)
            first = {"A": True, "B": True, "C": True}
            remaining = {"A": 0, "B": 0, "C": 0}
            cons_count = {"sp": 0, "act": 0, "pool": 0}
            for _, _, (chain, _, _, _) in PE_ORDER:
                remaining[chain] += 1
            for eng_key, k, (chain, src, lo, w) in PE_ORDER:
                ns = NSLOT[eng_key]
                tensor.wait_ge(fills[eng_key][k % ns], 16 * (k // ns + 1))
                sl = slots[eng_key][k % ns]
                remaining[chain] -= 1
                cons_count[eng_key] += 1
                nch = w // CHUNK
                # chain A stays open for the DVE contribution matmul
                closing = remaining[chain] == 0 and not (chain == "A" and N_DVE_ACT > 0)
                for c in range(nch):
                    mm = nc.tensor.matmul(
                        psm[chain][:, 0:min(w, CHUNK)] if w < CHUNK else psm[chain][:],
                        sel_ap,
                        sl[:, c * CHUNK:min((c + 1) * CHUNK, w)].bitcast(
                            F32 if FP32MM else F32R),
                        start=(first[chain] and c == 0),
                        stop=(closing and c == nch - 1),
                    )
                    if c == nch - 1:
                        mm.then_inc(s_cons[eng_key], 1)
                if remaining[chain] == 0:
                    chain_close[chain] = (eng_key, cons_count[eng_key])
                first[chain] = False
                if chain == "A" and remaining[chain] == 0 and N_DVE_ACT > 0:
                    # close chain A with the DVE-accumulated contribution
                    tensor.wait_ge(s_dacc, 1)
                    nc.tensor.matmul(
                        psA[:, 0:1], sel_f32, dcol[:],
                        start=False, stop=True,
                    ).then_inc(s_aclose, 1)
            # MLP layer 1: ps1 = G_peer.T @ fC + G_own.T @ fB + G_rx.T @ rx_aug
            tensor.wait_ge(s_fC, 1)
            nc.tensor.matmul(ps1[:], g_peer, fC[:], start=True, stop=False)
            tensor.wait_ge(s_fB, 1)
            nc.tensor.matmul(ps1[:], g_own, fB[:], start=False, stop=False)
            tensor.wait_ge(s_rx, 16)
            nc.tensor.matmul(
                ps1[:], g_rx, rx_aug[:], start=False, stop=True
            ).then_inc(s_pe1, 1)
            # MLP layer 2 logits
            tensor.wait_ge(s_haug, 1)
            nc.tensor.matmul(
                ps2[:], h_aug[:], w2ta, start=True, stop=True
            ).then_inc(s_pe2, 1)

        @block.vector
        def _(vector):
            vector.memset(rx_aug[64:65, :], 1.0).then_inc(s_ms, 1)
            vector.memset(h_aug[64:65, :], 1.0).then_inc(s_ms, 1)
            # dedicated-slot prefix tiles
            for i, (chain, src, lo, w) in enumerate(DVE_RECS):
                j = i % N_DVE_SLOT
                vector.wait_ge(s_fdve[j], 16 * (i // N_DVE_SLOT + 1))
                vector.reduce_sum(
                    dacc[:, i:i + 1], sl_dve[j][:, 0:w], axis=mybir.AxisListType.X
                ).then_inc(s_cdve, 1)
            if N_DVE_ACT > 0:
                vector.drain()
                vector.reduce_sum(
                    dcol[:], dacc[:, 0:N_DVE_ACT], axis=mybir.AxisListType.X
                ).then_inc(s_dacc, 1)
            # fold A -> CC payload
            if N_DVE_ACT > 0:
                vector.wait_ge(s_aclose, 1)
            else:
                eng, cnt = chain_close["A"]
                vector.wait_ge(s_cons[eng], cnt)
            vector.reduce_sum(
                partA[:], psA[:], axis=mybir.AxisListType.X
            ).then_inc(s_fA, 1)
            # folds C then B (chain C closes first)
            eng, cnt = chain_close["C"]
            vector.wait_ge(s_cons[eng], cnt)
            vector.reduce_sum(
                fC[:], psC[:], axis=mybir.AxisListType.X
            ).then_inc(s_fC, 1)
            eng, cnt = chain_close["B"]
            vector.wait_ge(s_cons[eng], cnt)
            vector.reduce_sum(
                fB[:], psB[:], axis=mybir.AxisListType.X
            ).then_inc(s_fB, 1)
            # leaky relu on ps1 (bias already folded in)
            vector.wait_ge(s_pe1, 1)
            vector.tensor_scalar_mul(h_sc[:], ps1[:], LEAKY_SLOPE)
            vector.drain()
            vector.tensor_max(h_aug[0:64, :], ps1[:], h_sc[:])
            vector.drain().then_inc(s_haug, 1)
            # top-8 / rank select on logits
            vector.wait_ge(s_pe2, 1)
            vector.tensor_copy(s_sb[:], ps2[:])
            vector.drain()
            vector.max(mx8[:], s_sb[:])
            vector.drain()
            vector.max_index(ix8[:], mx8[:], s_sb[:])
            vector.drain()
            vector.tensor_tensor(ixm[:], ix8[:], rsel, op=mybir.AluOpType.mult)
            vector.drain()
            with nc.allow_low_precision(reason="uint32 index ops are exact"):
                vector.tensor_reduce(
                    ixv[:], ixm[:], axis=mybir.AxisListType.X,
                    op=mybir.AluOpType.add,
                ).then_inc(s_top, 1)

    return nc


def make_consts(W1, b1, W2, b2, h):
    """Pack per-core constants: (cst [128, NCONST], icst [128, NICONST])."""
    cst = np.zeros((128, NCONST), np.float32)
    icst = np.zeros((128, NICONST), np.float32)
    inv_s = np.float32(1.0 / S)
    cst[0:64, _G_RX:_G_RX + 64] = (W1.T * inv_s).astype(np.float32)
    cst[64, _G_RX:_G_RX + 64] = b1
    cst[0:32, _G_OWN:_G_OWN + 64] = (W1.T[32 * h:32 * h + 32] * inv_s)
    cst[0:32, _G_PEER:_G_PEER + 64] = (W1.T[32 * (1 - h):32 * (1 - h) + 32] * inv_s)
    cst[0:64, _W2TA:_W2TA + 64] = W2.T
    cst[64, _W2TA:_W2TA + 64] = b2
    sel = np.zeros((128, 32), np.float32)
    sel[np.arange(128), np.arange(128) // 4] = 1.0
    cst[:, _SEL:_SEL + 32] = sel
    # gather indices: gather g reads xqv rows (128*hh + p)*32 + lo//TW
    gidx = np.zeros((128, 8 * max(N_GATH, 1)), np.int16)
    for g, (kind, idx, (chain, src, lo, w)) in enumerate(POOL_GATHERS):
        if NOGATHER:
            continue
        assert w == TW and lo % TW == 0, (lo, w)
        hh = h if chain in ("A", "B") else 1 - h
        j = lo // TW
        for i in range(128):
            p16, col = i % 16, i // 16
            gidx[p16, 8 * g + col] = (128 * hh + i) * (PP // TW) + j
    icst[:, _GIDX:_GIDX + 4 * N_GATH] = (
        gidx[:, 0:8 * N_GATH].view(np.float32).reshape(128, 4 * N_GATH)
    )
    rsel = np.zeros((1, 8), np.uint32)
    rsel[0, h] = 2048
    icst[0:1, _RSEL:_RSEL + 8] = rsel.view(np.float32)
    return cst, icst


def make_in_maps(x, W1, b1, W2, b2, n_cores=N_CORES):
    b_sz = x.shape[0]
    x2 = np.ascontiguousarray(x.reshape(b_sz, C, S))
    in_maps = []
    for i in range(n_cores):
        b_i, h_i = i // 2, i % 2
        xq = x2[b_i].reshape(256, PP)           # zero-copy view
        xr = xq[128 * h_i:128 * (h_i + 1)]      # contiguous view
        xp = xq[128 * (1 - h_i):128 * (2 - h_i)]
        cst_i, icst_i = make_consts(W1, b1, W2, b2, h_i)
        in_maps.append({
            "xr": xr, "xp": xp, "xq": xq, "cst": cst_i, "icst": icst_i,
        })
    return in_maps


def assemble_output(results, b_sz=B):
    d = h = w = 64
    out = np.empty((b_sz, R, d, h, w), np.float32)
    for i, res in enumerate(results):
        b_i, r_i = i // 2, i % 2
        out[b_i, r_i] = res["out"].reshape(d, h, w)
    return out


def kernel(x, W1, b1, W2, b2):
    x = np.asarray(x, dtype=np.float32)
    W1 = np.asarray(W1, dtype=np.float32)
    b1 = np.asarray(b1, dtype=np.float32)
    W2 = np.asarray(W2, dtype=np.float32)
    b2 = np.asarray(b2, dtype=np.float32)

    nc = build_bass()
    in_maps = make_in_maps(x, W1, b1, W2, b2)
    res = run_bass_kernel_spmd(nc, in_maps, list(range(N_CORES)))
    return assemble_output(res.results)


if __name__ == "__main__":
    rng = np.random.default_rng(0)
    x = rng.standard_normal((B, C, 64, 64, 64), dtype=np.float32)
    W1 = rng.standard_normal((C, C), dtype=np.float32) / np.sqrt(C)
    b1 = rng.standard_normal(C, dtype=np.float32) * 0.01
    W2 = rng.standard_normal((C, C), dtype=np.float32) / np.sqrt(C)
    b2 = rng.standard_normal(C, dtype=np.float32) * 0.01
    out = kernel(x=x, W1=W1, b1=b1, W2=W2, b2=b2)
    print(out.shape, out.dtype)
